# revision 117
# baseline (speedup 1.0000x reference)
"""Trainium2 Bass kernel for NodeNetworkG GNN message passing.

Algorithm (8 NeuronCores, SPMD, no collectives):
  - Nodes are sorted by total degree and dealt round-robin to 8 cores; each
    core owns ~1/8 of nodes and edges for both aggregation directions.
  - Per direction, destinations are packed into 128-node blocks (sorted by
    that direction's degree). Each edge is assigned to one of 8 "classes"
    (4 node-id range chunks + 4 mod-4 decimated copies) via 2-choice load
    balancing, so every gather instruction's int16 indices address a
    25088-row fp16 table (dma_gather is limited to 1024 indices/instruction).
  - Bulk tier: per (block, class) band with T slots per destination (T chosen
    per band to trade slot padding vs overflow, quantized over block groups).
    Batched dma_gather (96B rows from 256B-stride tables, 8192 idx/instr with
    single_packet=False), ACT-engine weight expansion + packed DVE multiply
    (2x mode) and uniform-T fold in fp16, accumulated into [128, nb*48]
    accumulators.
  - Overflow tier (edges beyond T): class-grouped dma_gather at full packing,
    weight-multiply, then dma_scatter_add into a DRAM accumulator (rank-sliced
    windows keep destinations unique per scatter instruction).
  - mo is realigned to mi-local node order via the DRAM accumulator and one
    dma_gather. Node-wise MLP per 128-node block: PE transpose to
    feature-major, two fp16 matmuls with tanh (ACT bias), written
    feature-major; host restores order.
"""

import numpy as np

P = 128
NCORES = 8
DIN = 48
DHID = 128
NPAD = 100352
NLOC = NPAD // NCORES      # 12544
NB = NLOC // P             # 98
CHUNK = NPAD // 4          # 25088
NCLS = 8
GMAX = 8192                # dma_gather / dma_scatter_add index limit
                           # (single_packet=False; >8192 wedges the ucode)
GCOLS = GMAX // P          # 64 columns per gather instruction
TILE_COLS = 64             # bulk G-tile columns (1 gather instr)
NRANGE = 6                 # mi accumulator ranges for MLP pipelining
QUANT_LAMBDA = 2           # DP grouping: column-cost of an extra fold group
COST_SLOT = 1.0            # rel cost of one bulk slot (Pool+DMA+DVE)
COST_OVF = 1.6             # rel cost of one overflow edge

USE_ACC2 = True           # dual scatter accumulators (WAW-chain breaking)
_PROG_CACHE: dict = {}
_EMIT_LOG: list = []


# --------------------------------------------------------------------------
# host prep
# --------------------------------------------------------------------------

def _class_assign(dst, src):
    """2-choice greedy per-destination class balancing. Returns cls [E]."""
    E = len(dst)
    cA = src // CHUNK
    cB = 4 + (src % 4)
    sortp = np.argsort(dst, kind="stable")
    degd = np.bincount(dst, minlength=NPAD)
    first = np.cumsum(degd) - degd
    srank_d = np.arange(E) - first[dst[sortp]]
    loads = np.zeros((NPAD, NCLS), np.int32)
    cls = np.empty(E, np.int64)
    maxr = int(srank_d.max()) if E else 0
    for r in range(maxr + 1):
        sel = sortp[srank_d == r]
        if len(sel) == 0:
            continue
        d = dst[sel]
        la = loads[d, cA[sel]]
        lb = loads[d, cB[sel]]
        pick_b = lb < la
        c = np.where(pick_b, cB[sel], cA[sel])
        loads[d, c] += 1
        cls[sel] = c
    return cls


def _quantize_groups(T_col, cuts):
    """DP: partition blocks into contiguous groups (respecting forced cuts),
    pad T to group max. Returns [(b0, nbs, Tq)] with Tq>0 only."""
    groups = []
    for ci in range(len(cuts) - 1):
        lo, hi = cuts[ci], cuts[ci + 1]
        n = hi - lo
        if n <= 0:
            continue
        seg = T_col[lo:hi]
        INF = 1 << 60
        best = [INF] * (n + 1)
        prev = [0] * (n + 1)
        best[0] = 0
        for j in range(1, n + 1):
            m = 0
            ssum = 0
            for i in range(j - 1, -1, -1):
                if seg[i] > m:
                    m = seg[i]
                ssum += seg[i]
                cost = best[i] + (m * (j - i) - ssum) + QUANT_LAMBDA
                if cost < best[j]:
                    best[j] = cost
                    prev[j] = i
        bounds = []
        j = n
        while j > 0:
            bounds.append((prev[j], j))
            j = prev[j]
        for i, j in reversed(bounds):
            Tq = int(seg[i:j].max())
            if Tq > 0:
                groups.append((lo + i, j - i, Tq))
    return groups


def _wrap_idx(vals16, pos):
    """Scatter int16 values into the wrapped-by-16, 8x-replicated layout.
    vals16 [n], pos [n] list positions. Returns writes for a [16, L] array:
    (rows, cols)."""
    return (pos % 16).astype(np.int64), pos // 16


def _build_direction(dst, src, w, deg, core, ranges):
    """Full per-direction layout: bulk bands + overflow lists."""
    loc = np.empty(NPAD, np.int64)
    nodes_by_core = []
    for k in range(NCORES):
        nodes_k = np.where(core == k)[0]
        # ascending degree: the last tiles are high-T bands spanning FEW
        # blocks, so late MLP ranges complete incrementally, not all at once
        lk = nodes_k[np.argsort(deg[nodes_k], kind="stable")]
        loc[lk] = np.arange(NLOC)
        nodes_by_core.append(lk)
    blk = loc // P
    part = loc % P

    cls = _class_assign(dst, src)
    E = len(dst)
    cA = src // CHUNK
    cB = 4 + (src % 4)
    cuts = sorted(set([0, NB] + list(ranges)))
    nodes = np.arange(NPAD)

    def _rank_and_T(cls):
        # per-edge rank within (dest, class)
        key = dst * NCLS + cls
        sortp = np.argsort(key, kind="stable")
        ks = key[sortp]
        cnt = np.bincount(ks, minlength=NPAD * NCLS)
        firstk = np.cumsum(cnt) - cnt
        srank = np.empty(E, np.int64)
        srank[sortp] = np.arange(E) - firstk[ks]
        loads = cnt.reshape(NPAD, NCLS)
        # T*: minimize 128*T*COST_SLOT + unified_ovf(T)*COST_OVF
        maxl = int(loads.max())
        hist = np.zeros((NCORES, NB, NCLS, maxl + 1), np.int64)
        for c in range(NCLS):
            np.add.at(hist, (core[nodes], blk[nodes], c, loads[nodes, c]), 1)
        cum = hist.cumsum(axis=3)
        total = cum[..., -1:]
        abovec = (total - cum)  # [..., lv] = #dests with load > lv
        suf = abovec[..., ::-1].cumsum(axis=3)[..., ::-1]
        ovf_unified = suf.max(axis=0)  # [NB, NCLS, maxl+1]
        Tcost = (128 * np.arange(maxl + 1)[None, None, :] * COST_SLOT
                 + ovf_unified * COST_OVF)
        Tstar = Tcost.argmin(axis=2)  # [NB, NCLS]
        Tstar = np.minimum(Tstar, TILE_COLS)
        groups_per_cls = [_quantize_groups(Tstar[:, c], cuts)
                          for c in range(NCLS)]
        Tq = np.zeros((NB, NCLS), np.int64)
        for c in range(NCLS):
            for b0, nbs, t in groups_per_cls[c]:
                Tq[b0 : b0 + nbs, c] = t
        return srank, loads, Tq, groups_per_cls

    def _ranked_move(sel, tgt_key, budget):
        """Rank sel (edge ids) within tgt_key groups; return those whose rank
        fits the per-group budget."""
        oord = sel[np.argsort(tgt_key, kind="stable")]
        oks = np.sort(tgt_key, kind="stable")
        ocnt = np.bincount(oks, minlength=NPAD * NCLS)
        ofirst = np.cumsum(ocnt) - ocnt
        orank = np.arange(len(oord)) - ofirst[oks]
        return oord[orank < budget[oks]]

    def _rebalance(cls, Tq, npass=10):
        # Fill free bulk slots with overflow edges. 1-hop: move an overflow
        # edge to its other class choice if free. 2-hop: vacate a bulk
        # resident (whose other choice has room) from a bin that overflow
        # edges want, then fill on the next pass.
        capf = Tq[blk]  # [NPAD, NCLS]
        for _ in range(npass):
            key = dst * NCLS + cls
            sortp = np.argsort(key, kind="stable")
            ks = key[sortp]
            cnt = np.bincount(ks, minlength=NPAD * NCLS)
            firstk = np.cumsum(cnt) - cnt
            srank = np.empty(E, np.int64)
            srank[sortp] = np.arange(E) - firstk[ks]
            loads = cnt.reshape(NPAD, NCLS)
            free = np.maximum(capf - loads, 0).reshape(-1)
            over = srank >= capf[dst, cls]
            other = np.where(cls == cA, cB, cA)
            osel = np.where(over)[0]
            if len(osel) == 0:
                break
            okey = dst[osel] * NCLS + other[osel]
            movers = _ranked_move(osel, okey, free)
            if len(movers):
                cls[movers] = other[movers]
                continue
            # 2-hop vacate
            demand = np.bincount(okey, minlength=NPAD * NCLS)
            res = np.where(~over)[0]
            rkey = dst[res] * NCLS + cls[res]
            ekey = dst[res] * NCLS + other[res]
            cand = res[(demand[rkey] > 0) & (free[ekey] > 0)]
            if len(cand) == 0:
                break
            # cap by free at the exit bin
            m1 = _ranked_move(cand, dst[cand] * NCLS + other[cand], free)
            if len(m1) == 0:
                break
            # cap by demand at the vacated bin
            m2 = _ranked_move(m1, dst[m1] * NCLS + cls[m1], demand)
            if len(m2) == 0:
                break
            cls[m2] = other[m2]
        return cls

    srank, loads, Tq, groups_per_cls = _rank_and_T(cls)
    for _ in range(2):
        cls = _rebalance(cls, Tq)
        srank, loads, Tq, groups_per_cls = _rank_and_T(cls)

    # ---- bulk tile packing ----
    # Per class: pack that class's block runs into tiles of <= TILE_COLS.
    # Tiles are then interleaved across classes by block progress (emission
    # order) and col0 assigned IN EMISSION ORDER so device idx loads are
    # contiguous and chunkable.
    cls_tiles = {c: [] for c in range(NCLS)}
    for c in range(NCLS):
        runs = []
        for b0, nbs, t in groups_per_cls[c]:
            nmax = max(1, TILE_COLS // t)
            j = 0
            while j < nbs:
                nn = min(nmax, nbs - j)
                runs.append((b0 + j, nn, t))
                j += nn
        cur = None
        for b0, nbs, t in runs:
            need = nbs * t
            if cur is None or cur["cols"] + need > TILE_COLS:
                if cur is not None:
                    cls_tiles[c].append(cur)
                cur = dict(cls=c, col0=-1, cols=0, entries=[])
            cur["entries"].append((t, b0, nbs, cur["cols"]))
            cur["cols"] += need
        if cur is not None:
            cls_tiles[c].append(cur)
    # interleave tiles by first-block progress, then assign col0 sequentially
    tiles = sorted(
        (t for c in range(NCLS) for t in cls_tiles[c]),
        key=lambda t: (t["entries"][0][1], t["cls"]),
    )
    colstart = np.full((NB, NCLS), -1, np.int64)
    col = 0
    for t in tiles:
        t["col0"] = col
        for s, b0, nbs, cbase in t["entries"]:
            for i in range(nbs):
                colstart[b0 + i, t["cls"]] = col + cbase + i * s
        col += t["cols"]
    CT = col

    percol_col0 = np.zeros(max(CT, 1), np.int64)
    for t in tiles:
        percol_col0[t["col0"] : t["col0"] + t["cols"]] = t["col0"]

    is_bulk = srank < Tq[blk[dst], cls]

    # class-local index values
    val = np.where(cls < 4, src - (cls * CHUNK), src // 4).astype(np.int16)

    eb = np.where(is_bulk)[0]
    d = dst[eb]
    k_e = core[d]
    b_e = blk[d]
    p_e = part[d]
    colg = colstart[b_e, cls[eb]] + srank[eb]
    col0 = percol_col0[colg]
    g_t = (colg - col0) * P + p_e
    ic = col0 * 8 + g_t // 16
    ir = g_t % 16

    idx16 = np.zeros((NCORES, 16, max(CT, 1) * 8), np.int16)
    idx16[k_e, ir, ic] = val[eb]
    idx_arr = np.tile(idx16, (1, 8, 1))
    w_arr = np.zeros((NCORES, P, max(CT, 1)), np.float16)
    w_arr[k_e, p_e, colg] = w[eb].astype(np.float16)

    # ---- overflow lists ----
    # class-major layout with PER-CLASS rank: crank = srank - cap (rank of
    # the edge within its (dest, class) beyond the bulk cap). Segments are
    # (class, crank); within a segment every dest appears at most once, so
    # each segment is one dest-unique dma_scatter_add. All segments of a
    # class are contiguous, so gathers merge to ~one per class.
    eo = np.where(~is_bulk)[0]
    if len(eo):
        ecls = cls[eo]
        crank = srank[eo] - Tq[blk[dst[eo]], ecls]
        NRK = int(crank.max()) + 1
        segc = np.zeros((NCORES, NCLS, NRK), np.int64)
        np.add.at(segc, (core[dst[eo]], ecls, crank), 1)
        useg = segc.max(axis=0)  # unified [NCLS, NRK]
        # pad to 128: gather lists restart at partition 0 per instruction
        useg = -(-useg // P) * P
    else:
        NRK = 1
        useg = np.zeros((NCLS, NRK), np.int64)
    seg_off = np.zeros((NCLS, NRK), np.int64)
    posn = 0
    segments = []   # (cls, pos0, n)
    for c in range(NCLS):
        for r in range(NRK):
            n = int(useg[c, r])
            if n == 0:
                continue
            assert n <= GMAX, "overflow segment exceeds one gather window"
            seg_off[c, r] = posn
            segments.append((c, posn, n))
            posn += n
    OVT = posn // P  # overflow columns
    # windows: pack consecutive segments into <= GMAX; per window the
    # device does per-class gathers, one multiply, and per-segment scatters
    ov_windows = []   # (pos0, n, [(spos, sn) scatters])
    ov_gathers = []   # (cls, pos0, n) per gather instruction (class-pure)
    cur = None
    for c, p0, n in segments:
        new_window = cur is None or cur[1] + n > GMAX
        if new_window:
            if cur is not None:
                ov_windows.append(tuple(cur))
            cur = [p0, 0, []]
        cur[2].append((p0, n))
        if (not new_window) and ov_gathers and ov_gathers[-1][0] == c \
                and ov_gathers[-1][1] + ov_gathers[-1][2] == p0:
            ov_gathers[-1] = (c, ov_gathers[-1][1], ov_gathers[-1][2] + n)
        else:
            ov_gathers.append((c, p0, n))
        cur[1] += n
    if cur is not None:
        ov_windows.append(tuple(cur))

    ov_idx16 = np.zeros((NCORES, 16, max(OVT, 1) * 8), np.int16)
    ov_sidx16 = np.full((NCORES, 16, max(OVT, 1) * 8), NLOC, np.int16)
    ov_w = np.zeros((NCORES, P, max(OVT, 1)), np.float16)
    if len(eo):
        # per-core position within segment: stable order by (core, cls, crank)
        okey = (core[dst[eo]] * NCLS + ecls) * NRK + crank
        osort = np.argsort(okey, kind="stable")
        oks = okey[osort]
        ocnt = np.bincount(oks, minlength=NCORES * NCLS * NRK)
        ofirst = np.cumsum(ocnt) - ocnt
        opos = np.empty(len(eo), np.int64)
        opos[osort] = np.arange(len(eo)) - ofirst[oks]
        e = eo
        posg = seg_off[ecls, crank] + opos  # list position
        kk = core[dst[e]]
        ov_idx16[kk, posg % 16, (posg // 16)] = val[e]
        ov_sidx16[kk, posg % 16, (posg // 16)] = loc[dst[e]].astype(np.int16)
        ov_w[kk, posg % P, posg // P] = w[e].astype(np.float16)
    ov_idx = np.tile(ov_idx16, (1, 8, 1))
    ov_sidx = np.tile(ov_sidx16, (1, 8, 1))

    return dict(
        loc=loc, blk=blk, part=part, nodes_by_core=nodes_by_core,
        tiles=tiles, CT=CT, idx_arr=idx_arr, w_arr=w_arr, Tq=Tq,
        OVT=OVT, ov_idx=ov_idx, ov_sidx=ov_sidx, ov_w=ov_w,
        ov_windows=ov_windows, ov_gathers=ov_gathers,
    )


def _host_prep(x, edge_index, edge_attr):
    N = x.shape[0]
    row = np.asarray(edge_index[0]).astype(np.int64)
    col = np.asarray(edge_index[1]).astype(np.int64)
    w = np.asarray(edge_attr, dtype=np.float32).reshape(-1)

    deg_in = np.bincount(col, minlength=NPAD)
    deg_out = np.bincount(row, minlength=NPAD)
    order = np.argsort(-(deg_in + deg_out), kind="stable")
    rank = np.empty(NPAD, np.int64)
    rank[order] = np.arange(NPAD)
    core = (rank % NCORES).astype(np.int64)

    ranges = [20, 40, 58, 74, 90]

    dmi = _build_direction(col, row, w, deg_in, core, ranges)
    dmo = _build_direction(row, col, w, deg_out, core, [])

    xf = np.zeros((NPAD, DIN), np.float32)
    xf[:N] = np.asarray(x, np.float32)
    x16 = xf.astype(np.float16)
    tabs = np.zeros((NCLS, CHUNK, 128), np.float16)
    for c in range(4):
        tabs[c, :, :DIN] = x16[c * CHUNK : (c + 1) * CHUNK]
    for r in range(4):
        tabs[4 + r, :, :DIN] = x16[r::4]

    x_own = np.zeros((NCORES, P, NB * DIN), np.float16)
    for k in range(NCORES):
        lk = dmi["nodes_by_core"][k]
        xv = x16[lk]
        x_own[k] = (
            xv.reshape(NB, P, DIN).transpose(1, 0, 2).reshape(P, NB * DIN)
        )

    realign = np.zeros((NCORES, 16, NLOC // 16), np.int16)
    g = np.arange(NLOC)
    for k in range(NCORES):
        lk = dmi["nodes_by_core"][k]
        vals = dmo["loc"][lk].astype(np.int16)
        realign[k, g % 16, g // 16] = vals
    realign = np.tile(realign, (1, 8, 1))

    return dict(
        N=N, core=core, dmi=dmi, dmo=dmo, tabs=tabs, x_own=x_own,
        realign=realign,
    )


# --------------------------------------------------------------------------
# numpy emulation (layout validation)
# --------------------------------------------------------------------------

def _emulate_agg(meta, direction):
    """Emulate both tiers -> acc [NCORES, 128, NB*48] fp32 in local order."""
    d = meta[direction]
    tabs = meta["tabs"]
    CT = d["CT"]
    acc = np.zeros((NCORES, P, NB, DIN), np.float32)
    colcls = np.zeros(max(CT, 1), np.int64)
    colblk = np.zeros(max(CT, 1), np.int64)
    col0a = np.zeros(max(CT, 1), np.int64)
    for t in d["tiles"]:
        col0a[t["col0"] : t["col0"] + t["cols"]] = t["col0"]
        for s, b0, nbs, cbase in t["entries"]:
            for i in range(nbs):
                c0 = t["col0"] + cbase + i * s
                colcls[c0 : c0 + s] = t["cls"]
                colblk[c0 : c0 + s] = b0 + i
    for k in range(NCORES):
        if CT:
            idx = d["idx_arr"][k]
            wv = d["w_arr"][k]
            cols = np.arange(CT)
            pp = np.arange(P)
            g_t = (cols[None, :] - col0a[None, :]) * P + pp[:, None]
            ic = col0a[None, :] * 8 + g_t // 16
            ir = g_t % 16
            vals = idx[ir, ic].astype(np.int64)
            gathered = tabs[colcls[None, :].repeat(P, 0), vals][:, :, :DIN]
            prod = gathered.astype(np.float16) * wv[:, :, None]
            np.add.at(acc[k], (slice(None), colblk), prod.astype(np.float32))
        # overflow
        OVT = d["OVT"]
        if OVT:
            oi = d["ov_idx"][k]
            os_ = d["ov_sidx"][k]
            ow = d["ov_w"][k]
            # reconstruct per-position
            ocls = np.zeros(OVT * P, np.int64)
            for c, pos0, n in d["ov_gathers"]:
                ocls[pos0 : pos0 + n] = c
            g = np.arange(OVT * P)
            vals = oi[g % 16, g // 16].astype(np.int64)
            sidx = os_[g % 16, g // 16].astype(np.int64)
            wvals = ow[g % P, g // P].astype(np.float16)
            gath = tabs[ocls, vals][:, :DIN].astype(np.float16)
            prod = (gath * wvals[:, None]).astype(np.float32)
            sel = sidx < NLOC
            tgt = sidx[sel]
            np.add.at(acc[k], (tgt % P, tgt // P), prod[sel])
    return acc.reshape(NCORES, P, NB * DIN)


def _emulate(meta, W1, b1, W2, b2):
    mi = _emulate_agg(meta, "dmi")
    mo = _emulate_agg(meta, "dmo")
    out = np.zeros((NPAD, DHID), np.float32)
    for k in range(NCORES):
        lk_i = meta["dmi"]["nodes_by_core"][k]
        lk_o = meta["dmo"]["nodes_by_core"][k]
        mi_k = mi[k].reshape(P, NB, DIN).transpose(1, 0, 2).reshape(NLOC, DIN)
        mo_k = mo[k].reshape(P, NB, DIN).transpose(1, 0, 2).reshape(NLOC, DIN)
        mo_full = np.zeros((NLOC, DIN), np.float32)
        mo_full[:] = mo_k  # mo-local order
        mo2_k = mo_full[meta["dmo"]["loc"][lk_i]]
        x_k = meta["x_own"][k].reshape(P, NB, DIN).transpose(1, 0, 2).reshape(
            NLOC, DIN
        ).astype(np.float32)
        M = np.concatenate([mi_k, mo2_k, x_k], axis=1)
        h = np.tanh(M @ W1.T + b1)
        out[lk_i] = np.tanh(h @ W2.T + b2)
    return out[: meta["N"]]


# --------------------------------------------------------------------------
# device program
# --------------------------------------------------------------------------

def _dma_gather96(gp, mybir, out_ap, in_ap, idxs_ap, num_idxs, reg=None):
    from concourse.bass import exact_div

    elem_step = in_ap.ap[0][0]
    stride_bytes = elem_step * mybir.dt.size(in_ap.dtype)
    return gp.add_instruction(
        mybir.InstDMAGatherAnt(
            name=gp.bass.get_next_instruction_name(),
            ins=[
                *gp.lower_ap_dma(in_ap, for_custom_bir_dma=True),
                gp.lower_ap(idxs_ap),
                gp.lower_val_access(
                    gp.to_reg(num_idxs) if reg is None else reg
                ),
            ],
            outs=[gp.lower_ap(out_ap)],
            transpose=False,
            num_idxs=num_idxs,
            elem_size=DIN,
            stride_bytes_256=exact_div(stride_bytes, 256),
            gen_mode=0,
            single_packet=False,
            queue_num=0,
            sbuf_tokens_per_rank=0,
            sbuf_free_dim_per_rank=0,
            sbuf_free_dim_pad_per_rank=0,
            sbuf_byte_offset=0,
        )
    )


def _build_program(meta):
    import concourse.bacc as bacc
    import concourse.bass as bass
    import concourse.mybir as mybir
    import concourse.tile as tile
    from concourse.masks import make_identity

    f32 = mybir.dt.float32
    f16 = mybir.dt.float16
    i16 = mybir.dt.int16

    dmi, dmo = meta["dmi"], meta["dmo"]
    CTI, CTO = max(dmi["CT"], 1), max(dmo["CT"], 1)
    OVI, OVO = max(dmi["OVT"], 1), max(dmo["OVT"], 1)

    nc = bacc.Bacc(
        "TRN2",
        target_bir_lowering=False,
        debug=False,
        num_devices=NCORES,
        dynamic_dma_scratch_size=65536,
    )

    tabs_d = [
        nc.dram_tensor(f"tab{c}", [CHUNK, 128], f16, kind="ExternalInput")
        for c in range(NCLS)
    ]
    idx_mi_d = nc.dram_tensor("idx_mi", [P, CTI * 8], i16, kind="ExternalInput")
    idx_mo_d = nc.dram_tensor("idx_mo", [P, CTO * 8], i16, kind="ExternalInput")
    w_mi_d = nc.dram_tensor("w_mi", [P, CTI], f16, kind="ExternalInput")
    w_mo_d = nc.dram_tensor("w_mo", [P, CTO], f16, kind="ExternalInput")
    ovi_idx_d = nc.dram_tensor("ovi_idx", [P, OVI * 8], i16, kind="ExternalInput")
    ovi_sidx_d = nc.dram_tensor("ovi_sidx", [P, OVI * 8], i16, kind="ExternalInput")
    ovi_w_d = nc.dram_tensor("ovi_w", [P, OVI], f16, kind="ExternalInput")
    ovo_idx_d = nc.dram_tensor("ovo_idx", [P, OVO * 8], i16, kind="ExternalInput")
    ovo_sidx_d = nc.dram_tensor("ovo_sidx", [P, OVO * 8], i16, kind="ExternalInput")
    ovo_w_d = nc.dram_tensor("ovo_w", [P, OVO], f16, kind="ExternalInput")
    x_own_d = nc.dram_tensor("x_own", [P, NB * DIN], f16, kind="ExternalInput")
    realign_d = nc.dram_tensor("realign", [P, NLOC // 16], i16, kind="ExternalInput")
    mi_acc = nc.dram_tensor("mi_acc", [NLOC + P, 128], f16, kind="Internal")
    mo_acc = nc.dram_tensor("mo_acc", [NLOC + P, 128], f16, kind="Internal")
    # second accumulator per direction: consecutive dma_scatter_adds to one
    # tensor serialize on WAW semaphores (~3.3us each); alternating targets
    # breaks the false chain (adds commute)
    mi_acc2 = nc.dram_tensor("mi_acc2", [NLOC + P, 128], f16, kind="Internal")
    mo_acc2 = nc.dram_tensor("mo_acc2", [NLOC + P, 128], f16, kind="Internal")
    mi_accs = (mi_acc,) if not USE_ACC2 else (mi_acc, mi_acc2)
    mo_accs = (mo_acc,) if not USE_ACC2 else (mo_acc, mo_acc2)
    w1ta_d = nc.dram_tensor("w1ta", [DIN, DHID], f16, kind="ExternalInput")
    w1tb_d = nc.dram_tensor("w1tb", [DIN, DHID], f16, kind="ExternalInput")
    w1tc_d = nc.dram_tensor("w1tc", [DIN, DHID], f16, kind="ExternalInput")
    w2t_d = nc.dram_tensor("w2t", [DHID, DHID], f16, kind="ExternalInput")
    b1_d = nc.dram_tensor("b1", [DHID, 1], f32, kind="ExternalInput")
    b2_d = nc.dram_tensor("b2", [DHID, 1], f32, kind="ExternalInput")
    out_t = nc.dram_tensor("out_t", [P, NLOC], f16, kind="ExternalOutput")

    rbounds = [0, 20, 40, 58, 74, 90, NB]

    with tile.TileContext(nc) as tc:
        with (
            tc.tile_pool(name="const", bufs=1) as const,
            tc.tile_pool(name="gidx", bufs=6) as gidx,
            tc.tile_pool(name="gpool", bufs=3) as gpool,
            tc.tile_pool(name="wexp", bufs=3) as wexp,
            tc.tile_pool(name="mlp", bufs=4) as mlp,
            tc.tile_pool(name="ost", bufs=2) as ostp,
            tc.tile_pool(name="psT", bufs=4, space="PSUM") as psT,
            tc.tile_pool(name="psH", bufs=1, space="PSUM") as psH,
        ):
            _regs = {}

            def _greg(n):
                r = _regs.get(n)
                if r is None:
                    r = nc.gpsimd.to_reg(n)
                    _regs[n] = r
                return r

            w_mi_sb = const.tile([P, CTI], f16)
            nc.scalar.dma_start(w_mi_sb[:], w_mi_d[:])
            w_mo_sb = const.tile([P, CTO], f16)
            nc.scalar.dma_start(w_mo_sb[:], w_mo_d[:])
            ovi_w_sb = const.tile([P, OVI], f16)
            nc.scalar.dma_start(ovi_w_sb[:], ovi_w_d[:])
            ovo_w_sb = const.tile([P, OVO], f16)
            nc.scalar.dma_start(ovo_w_sb[:], ovo_w_d[:])
            realign_sb = const.tile([P, NLOC // 16], i16)
            nc.sync.dma_start(realign_sb[:], realign_d[:])
            w1ta_sb = const.tile([DIN, DHID], f16)
            nc.sync.dma_start(w1ta_sb[:], w1ta_d[:])
            w1tb_sb = const.tile([DIN, DHID], f16)
            nc.sync.dma_start(w1tb_sb[:], w1tb_d[:])
            w1tc_sb = const.tile([DIN, DHID], f16)
            nc.sync.dma_start(w1tc_sb[:], w1tc_d[:])
            w2t_sb = const.tile([DHID, DHID], f16)
            nc.sync.dma_start(w2t_sb[:], w2t_d[:])
            b1_sb = const.tile([DHID, 1], f32)
            nc.sync.dma_start(b1_sb[:], b1_d[:])
            b2_sb = const.tile([DHID, 1], f32)
            nc.sync.dma_start(b2_sb[:], b2_d[:])
            ident = const.tile([P, P], f16)
            make_identity(nc, ident[:])
            xo_sb = const.tile([P, NB * DIN], f16)
            nc.scalar.dma_start(xo_sb[:], x_own_d[:])

            mo_sb = const.tile([P, NB * DIN], f16)
            mo2_sb = const.tile([P, NB * DIN], f16)
            mi_rs = []
            for ri in range(NRANGE):
                mi_ri = const.tile([P, (rbounds[ri + 1] - rbounds[ri]) * DIN],
                                   f16, name=f"mi_r{ri}")
                mi_rs.append(mi_ri)
                nc.vector.memset(mi_ri[:], 0.0)
            zz = const.tile([P, DIN], f16)
            nc.vector.memset(mo_sb[:], 0.0)
            nc.vector.memset(zz[:], 0.0)
            # zero DRAM accumulators (NLOC rows from zeroed mo_sb; the dummy
            # padding row NLOC is never read back, but zero acc1's for safety)
            def zero_accs(accs, dummy):
                for acc_d in accs:
                    nc.scalar.dma_start(
                        acc_d[0:NLOC, 0:DIN].rearrange(
                            "(b p) f -> p b f", p=P
                        ),
                        mo_sb[:].rearrange("p (b f) -> p b f", f=DIN),
                    )
                for acc_d in dummy:
                    nc.scalar.dma_start(
                        acc_d[NLOC : NLOC + P, 0:DIN].rearrange(
                            "(b p) f -> p b f", p=P
                        ),
                        zz[:].rearrange("p (b f) -> p b f", f=DIN),
                    )
            zero_accs(mi_accs + mo_accs, (mi_acc, mo_acc))


            IDX_CHUNK_COLS = 4 * TILE_COLS
            # deferred overflow scatters, drained one per bulk tile (2+ tiles
            # after their window's multiply) so the Pool SEQ never bursts
            # scatters nor waits on a fresh DVE multiply
            ovctx = {}  # name -> {"pending": [(ready_at, win, args)], ...}
            tile_no = [0]
            win_no = [0]

            def drain_one():
                for ctx in ovctx.values():
                    if ctx["pending"] and ctx["pending"][0][0] <= tile_no[0]:
                        emit_overflow_s(ctx, ctx["pending"].pop(0)[2])
                        return

            def flush_ovf(name=None, upto_win=None):
                # OG/oi/os rings are 2-deep: before emitting global window k,
                # scatters of windows <= k-2 must be out (upto_win=k-2);
                # name-flush (merges) drains everything for that context
                for nm, ctx in ovctx.items():
                    if name is not None and nm != name:
                        continue
                    while ctx["pending"] and (
                        upto_win is None or ctx["pending"][0][1] <= upto_win
                    ):
                        emit_overflow_s(ctx, ctx["pending"].pop(0)[2])

            def preload_first_chunk(dirmeta, idx_d):
                # emit the first idx chunk's DMA ahead of the const/zeroing
                # stream so the very first gather is not queued behind it
                cur, ccols = [], 0
                for t in dirmeta["tiles"]:
                    if cur and ccols + t["cols"] > IDX_CHUNK_COLS:
                        break
                    cur.append(t)
                    ccols += t["cols"]
                base = cur[0]["col0"]
                tot = sum(t["cols"] for t in cur)
                idx_sb = gidx.tile([P, IDX_CHUNK_COLS * 8], i16,
                                   tag="gi", bufs=4)
                nc.sync.dma_start(
                    idx_sb[:, : tot * 8],
                    idx_d[:, base * 8 : (base + tot) * 8],
                )
                return idx_sb

            def emit_bulk(dirmeta, idx_d, w_sb, acc_of, only_range=None,
                          ovf=None, act_expand=True, pre=None):
                rlo, rhi = (0, NB) if only_range is None else only_range
                tl = [t for t in dirmeta["tiles"]
                      if rlo <= t["entries"][0][1] < rhi]
                wins = list(dirmeta["ov_windows"]) if ovf is not None else []
                step = max(1, len(tl) // (len(wins) + 1)) if wins else 1 << 30
                if ovf is not None:
                    ctx = ovctx.setdefault(
                        ovf[5], {"pending": [], "accs": ovf[4], "si": 0}
                    )
                chunks = []
                cur, ccols = [], 0
                for t in tl:
                    if cur and ccols + t["cols"] > IDX_CHUNK_COLS:
                        chunks.append(cur)
                        cur, ccols = [], 0
                    cur.append(t)
                    ccols += t["cols"]
                if cur:
                    chunks.append(cur)
                wi = 0
                i = 0
                for ci, ch in enumerate(chunks):
                    base = ch[0]["col0"]
                    tot = sum(t["cols"] for t in ch)
                    if ci == 0 and pre is not None:
                        idx_sb = pre
                    else:
                        idx_sb = gidx.tile([P, IDX_CHUNK_COLS * 8], i16,
                                           tag="gi", bufs=4)
                        nc.sync.dma_start(
                            idx_sb[:, : tot * 8],
                            idx_d[:, base * 8 : (base + tot) * 8],
                        )
                    for t in ch:
                        emit_one_bulk(dirmeta, t, idx_sb, base, w_sb, acc_of,
                                      act_expand)
                        tile_no[0] += 1
                        drain_one()
                        if wi < len(wins) and i % step == step - 1:
                            win_no[0] += 1
                            flush_ovf(upto_win=win_no[0] - 2)
                            G, sidx_sb, wpos, scats = emit_overflow_g(
                                ovf[0], ovf[1], ovf[2], ovf[3], wins[wi]
                            )
                            ctx["pending"].extend(
                                (tile_no[0] + 3, win_no[0],
                                 (G, sidx_sb, wpos, [s]))
                                for s in scats
                            )
                            wi += 1
                        i += 1
                while wi < len(wins):
                    win_no[0] += 1
                    flush_ovf(upto_win=win_no[0] - 2)
                    G, sidx_sb, wpos, scats = emit_overflow_g(
                        ovf[0], ovf[1], ovf[2], ovf[3], wins[wi]
                    )
                    ctx["pending"].extend(
                        (0, win_no[0], (G, sidx_sb, wpos, [s]))
                        for s in scats
                    )
                    wi += 1

            def emit_one_bulk(dirmeta, t, idx_sb, base, w_sb, acc_of,
                              act_expand=True):
                    _EMIT_LOG.append(f"B{t['cols']}")
                    cols = t["cols"]
                    c = t["cls"]
                    col0 = t["col0"]
                    ioff = (col0 - base) * 8
                    G = gpool.tile([P, TILE_COLS * DIN], f16, tag="G", bufs=6)
                    for q0 in range(0, cols, GCOLS):
                        qn = min(GCOLS, cols - q0)
                        _dma_gather96(
                            nc.gpsimd, mybir,
                            out_ap=G[:, q0 * DIN : (q0 + qn) * DIN].rearrange(
                                "p (c f) -> p c f", f=DIN
                            ),
                            in_ap=tabs_d[c][:, 0:DIN],
                            idxs_ap=idx_sb[
                                :, ioff + q0 * 8 : ioff + (q0 + qn) * 8
                            ],
                            num_idxs=qn * P, reg=_greg(qn * P),
                        )
                    g3 = G[:, : cols * DIN].rearrange("p (c f) -> p c f", f=DIN)
                    wv = w_sb[:, col0 : col0 + cols]
                    wb = bass.AP(
                        wv.tensor,
                        wv.offset,
                        [list(wv.ap[0]), list(wv.ap[1]), [0, DIN]],
                    )
                    if act_expand:
                        # expand weights on ACT so the DVE multiply is fully
                        # packed 2-byte (2x mode)
                        w48 = wexp.tile([P, TILE_COLS * DIN], f16, tag="W", bufs=4)
                        w3 = w48[:, : cols * DIN].rearrange(
                            "p (c f) -> p c f", f=DIN
                        )
                        nc.scalar.activation(
                            w3, wb, mybir.ActivationFunctionType.Copy
                        )
                        in1 = w3
                    else:
                        in1 = wb
                    nc.vector.tensor_tensor(
                        out=g3, in0=g3, in1=in1, op=mybir.AluOpType.mult
                    )
                    for s, b0, nbs, cbase in t["entries"]:
                        gg = G[
                            :, cbase * DIN : (cbase + nbs * s) * DIN
                        ].rearrange("p (b s f) -> p b s f", s=s, f=DIN)
                        ss = s
                        while ss > 1:
                            half = ss // 2
                            hi0 = ss - half
                            nc.vector.tensor_tensor(
                                out=gg[:, :, 0:half, :],
                                in0=gg[:, :, 0:half, :],
                                in1=gg[:, :, hi0:ss, :],
                                op=mybir.AluOpType.add,
                            )
                            ss = hi0
                        acc_sb, boff = acc_of(b0)
                        accv = acc_sb[
                            :, (b0 - boff) * DIN : (b0 - boff + nbs) * DIN
                        ].rearrange("p (b f) -> p b f", f=DIN)
                        nc.vector.tensor_tensor(
                            out=accv, in0=accv, in1=gg[:, :, 0, :],
                            op=mybir.AluOpType.add,
                        )

            def emit_overflow_g(dirmeta, oidx_d, osidx_d, ow_sb, window):
                _EMIT_LOG.append(f"W[{window[0]}]")
                gathers = dirmeta["ov_gathers"]
                wpos, wn, scats = window
                wc = wn // P
                gi = 0
                while gi < len(gathers) and gathers[gi][1] < wpos:
                    gi += 1
                idx_sb = gidx.tile([P, GCOLS * 8], i16, tag="oi", bufs=2)
                nc.sync.dma_start(
                    idx_sb[:, : wn // 16],
                    oidx_d[:, (wpos // 16) : (wpos + wn) // 16],
                )
                sidx_sb = gidx.tile([P, GCOLS * 8], i16, tag="os", bufs=2)
                nc.sync.dma_start(
                    sidx_sb[:, : wn // 16],
                    osidx_d[:, (wpos // 16) : (wpos + wn) // 16],
                )
                G = gpool.tile([P, GCOLS * DIN], f16, tag="OG", bufs=2)
                while gi < len(gathers) and gathers[gi][1] < wpos + wn:
                    c, pos0, n = gathers[gi]
                    lo = pos0 - wpos
                    _dma_gather96(
                        nc.gpsimd, mybir,
                        out_ap=G[
                            :, (lo // P) * DIN : ((lo + n) // P) * DIN
                        ].rearrange("p (c f) -> p c f", f=DIN),
                        in_ap=tabs_d[c][:, 0:DIN],
                        idxs_ap=idx_sb[:, lo // 16 : (lo + n) // 16],
                        num_idxs=n, reg=_greg(n),
                    )
                    gi += 1
                g3 = G[:, : wc * DIN].rearrange("p (c f) -> p c f", f=DIN)
                wv = ow_sb[:, wpos // P : wpos // P + wc]
                wb = bass.AP(
                    wv.tensor,
                    wv.offset,
                    [list(wv.ap[0]), list(wv.ap[1]), [0, DIN]],
                )
                w48 = wexp.tile([P, TILE_COLS * DIN], f16, tag="W", bufs=4)
                w3 = w48[:, : wc * DIN].rearrange("p (c f) -> p c f", f=DIN)
                nc.scalar.activation(
                    w3, wb, mybir.ActivationFunctionType.Copy
                )
                nc.vector.tensor_tensor(
                    out=g3, in0=g3, in1=w3, op=mybir.AluOpType.mult
                )
                return (G, sidx_sb, wpos, scats)

            def emit_overflow_s(ctx, pend):
                acc_d = ctx["accs"][ctx["si"] % len(ctx["accs"])]
                ctx["si"] += 1
                G, sidx_sb, wpos, scats = pend
                _EMIT_LOG.append(f"S{sum(n for _, n in scats)}")
                for spos, sn in scats:
                    lo = spos - wpos
                    nc.gpsimd.dma_scatter_add(
                        out_ap=acc_d[:, 0:DIN],
                        in_ap=G[
                            :, (lo // P) * DIN : ((lo + sn) // P) * DIN
                        ].rearrange("p (c f) -> p c f", f=DIN),
                        idxs_ap=sidx_sb[:, lo // 16 : (lo + sn) // 16],
                        num_idxs=sn,
                        num_idxs_reg=_greg(sn),
                        elem_size=DIN,
                        elem_step=128,
                        single_packet=False,
                    )

            def acc_mo(b0):
                return mo_sb, 0

            def acc_mi(b0):
                for ri in range(NRANGE):
                    if b0 < rbounds[ri + 1]:
                        return mi_rs[ri], rbounds[ri]
                raise AssertionError(b0)

            # ---- mo: bulk with overflow windows interleaved ----
            emit_bulk(dmo, idx_mo_d, w_mo_sb, acc_mo,
                      ovf=(dmo, ovo_idx_d, ovo_sidx_d, ovo_w_sb,
                           mo_accs, "mo"))

            def emit_mo_merge():
                # merge: mo_sb += mo_acc + mo_acc2 (chunked); write back
                for acc_d in mo_accs:
                    for c0 in range(0, NB, 32):
                        cn = min(32, NB - c0)
                        tmpc_full = gpool.tile([P, 32 * DIN], f16, tag="mr",
                                               bufs=2)
                        tmpc = tmpc_full[:, : cn * DIN]
                        nc.sync.dma_start(
                            tmpc[:].rearrange("p (b f) -> p b f", f=DIN),
                            acc_d[c0 * P : (c0 + cn) * P, 0:DIN].rearrange(
                                "(b p) f -> p b f", p=P
                            ),
                        )
                        nc.vector.tensor_tensor(
                            out=mo_sb[:, c0 * DIN : (c0 + cn) * DIN],
                            in0=mo_sb[:, c0 * DIN : (c0 + cn) * DIN],
                            in1=tmpc[:],
                            op=mybir.AluOpType.add,
                        )
                nc.sync.dma_start(
                    mo_acc[0:NLOC, 0:DIN].rearrange("(b p) f -> p b f", p=P),
                    mo_sb[:].rearrange("p (b f) -> p b f", f=DIN),
                )

            def emit_mlp(rlo, rhi, late=False):
                OG = 8
                for b0 in range(rlo, rhi, OG):
                    og = min(OG, rhi - b0)
                    os_ = ostp.tile([P, OG * P], f16, tag="oo")
                    hp = psH.tile([P, OG * P], f32, tag="hp")
                    for j in range(og):
                        b = b0 + j
                        mi_t, mi_b0 = acc_mi(b)
                        pA = psT.tile([DIN, 3 * P], f16, tag="pA")
                        for q, (src_sb, bb) in enumerate((
                            (mi_t, b - mi_b0),
                            (mo2_sb, b),
                            (xo_sb, b),
                        )):
                            nc.tensor.transpose(
                                pA[:, q * P : (q + 1) * P],
                                src_sb[:, bb * DIN : (bb + 1) * DIN],
                                ident[:],
                            )
                        mt = mlp.tile([DIN, 3 * P], f16, tag="mt")
                        # one wide PSUM->SBUF copy per block, alternating
                        # ACT/DVE
                        if b % 2 == 0:
                            nc.scalar.copy(out=mt[:], in_=pA[:])
                        else:
                            nc.vector.tensor_copy(out=mt[:], in_=pA[:])
                        for q, w1q in enumerate(
                            (w1ta_sb, w1tb_sb, w1tc_sb)
                        ):
                            nc.tensor.matmul(
                                hp[:, j * P : (j + 1) * P], w1q[:],
                                mt[:, q * P : (q + 1) * P],
                                start=(q == 0), stop=(q == 2),
                            )
                    hs = mlp.tile([P, OG * P], f16, tag="hs")
                    nc.scalar.activation(
                        hs[:, : og * P], hp[:, : og * P],
                        mybir.ActivationFunctionType.Tanh,
                        bias=b1_sb[:], scale=1.0,
                    )
                    op_ = psH.tile([P, OG * P], f32, tag="op")
                    for j in range(og):
                        nc.tensor.matmul(
                            op_[:, j * P : (j + 1) * P], w2t_sb[:],
                            hs[:, j * P : (j + 1) * P], start=True, stop=True
                        )
                    nc.scalar.activation(
                        os_[:, : og * P], op_[:, : og * P],
                        mybir.ActivationFunctionType.Tanh,
                        bias=b2_sb[:], scale=1.0,
                    )
                    nc.sync.dma_start(
                        out_t[:, b0 * P : (b0 + og) * P], os_[:, : og * P]
                    )

            def merge_and_mlp(ri):
                _EMIT_LOG.append(f"M{ri}")
                rlo, rhi = rbounds[ri], rbounds[ri + 1]
                # merge overflow acc for this range (all mi scatters must be
                # emitted by now)
                flush_ovf("mi")
                for acc_d in mi_accs:
                    tmpr_full = gpool.tile([P, 32 * DIN], f16, tag="mr",
                                           bufs=2)
                    tmpr = tmpr_full[:, : (rhi - rlo) * DIN]
                    nc.sync.dma_start(
                        tmpr[:].rearrange("p (b f) -> p b f", f=DIN),
                        acc_d[rlo * P : rhi * P, 0:DIN].rearrange(
                            "(b p) f -> p b f", p=P
                        ),
                    )
                    nc.vector.tensor_tensor(
                        out=mi_rs[ri][:],
                        in0=mi_rs[ri][:],
                        in1=tmpr[:],
                        op=mybir.AluOpType.add,
                    )
                emit_mlp(rlo, rhi, late=(ri >= 2))

            # bulk ranges with merge+MLP shifted one range later, so mi
            # overflow scatters get two ranges of bulk tiles to spread over
            for ri in range(NRANGE):
                rlo, rhi = rbounds[ri], rbounds[ri + 1]
                if ri == 0:
                    emit_bulk(dmi, idx_mi_d, w_mi_sb, acc_mi, (rlo, rhi),
                              ovf=(dmi, ovi_idx_d, ovi_sidx_d, ovi_w_sb,
                                   mi_accs, "mi"))
                else:
                    emit_bulk(dmi, idx_mi_d, w_mi_sb, acc_mi, (rlo, rhi),
                              act_expand=True)
                    if ri == 1:
                        # mo merge waits on ALL mo scatters; emitting it only
                        # after two mi ranges of bulk keeps the DVE/SP queues
                        # from head-of-line blocking on that barrier
                        flush_ovf("mo")
                        emit_mo_merge()
                        for g0 in range(0, NLOC, GMAX):
                            gn = min(GMAX, NLOC - g0)
                            _dma_gather96(
                                nc.gpsimd, mybir,
                                out_ap=mo2_sb[
                                    :, (g0 // P) * DIN
                                    : ((g0 + gn) // P) * DIN
                                ].rearrange("p (b f) -> p b f", f=DIN),
                                in_ap=mo_acc[:, 0:DIN],
                                idxs_ap=realign_sb[
                                    :, g0 // 16 : (g0 + gn) // 16
                                ],
                                num_idxs=gn, reg=_greg(gn),
                            )
                    merge_and_mlp(ri - 1)
            merge_and_mlp(NRANGE - 1)

    nc.compile()
    return nc


# --------------------------------------------------------------------------
# entry point
# --------------------------------------------------------------------------

def kernel(x, edge_index, edge_attr, W1, b1, W2, b2):
    x = np.asarray(x, np.float32)
    meta = _host_prep(x, edge_index, edge_attr)
    dmi, dmo = meta["dmi"], meta["dmo"]
    key = (meta["N"], dmi["CT"], dmo["CT"], dmi["OVT"], dmo["OVT"],
           tuple(t["col0"] for t in dmi["tiles"]),
           tuple(t["col0"] for t in dmo["tiles"]),
           tuple(dmi["ov_gathers"]), tuple(dmo["ov_gathers"]),
           tuple((p, n, tuple(s)) for p, n, s in dmi["ov_windows"]),
           tuple((p, n, tuple(s)) for p, n, s in dmo["ov_windows"]))
    if key not in _PROG_CACHE:
        _PROG_CACHE[key] = _build_program(meta)
    nc = _PROG_CACHE[key]

    W1 = np.asarray(W1, np.float32)
    W2 = np.asarray(W2, np.float32)
    b1v = np.asarray(b1, np.float32).reshape(DHID, 1)
    b2v = np.asarray(b2, np.float32).reshape(DHID, 1)
    w1t = np.ascontiguousarray(W1.T)
    w1ta = np.ascontiguousarray(w1t[:DIN]).astype(np.float16)
    w1tb = np.ascontiguousarray(w1t[DIN : 2 * DIN]).astype(np.float16)
    w1tc = np.ascontiguousarray(w1t[2 * DIN :]).astype(np.float16)
    w2t = np.ascontiguousarray(W2.T).astype(np.float16)

    in_maps = []
    for k in range(NCORES):
        m = {
            "idx_mi": dmi["idx_arr"][k],
            "idx_mo": dmo["idx_arr"][k],
            "w_mi": dmi["w_arr"][k],
            "w_mo": dmo["w_arr"][k],
            "ovi_idx": dmi["ov_idx"][k],
            "ovi_sidx": dmi["ov_sidx"][k],
            "ovi_w": dmi["ov_w"][k],
            "ovo_idx": dmo["ov_idx"][k],
            "ovo_sidx": dmo["ov_sidx"][k],
            "ovo_w": dmo["ov_w"][k],
            "x_own": meta["x_own"][k],
            "realign": meta["realign"][k],
            "w1ta": w1ta, "w1tb": w1tb, "w1tc": w1tc, "w2t": w2t,
            "b1": b1v, "b2": b2v,
        }
        for c in range(NCLS):
            m[f"tab{c}"] = meta["tabs"][c]
        in_maps.append(m)

    runner = _get_runner(nc)
    results = runner.run(in_maps)
    global _LAST
    _LAST = (nc, in_maps)

    out = np.empty((NPAD, DHID), np.float32)
    for k in range(NCORES):
        out[dmi["nodes_by_core"][k]] = results[k]["out_t"].T.astype(np.float32)
    return out[: meta["N"]]


_LAST = None
_RUNNER_CACHE: dict = {}


class _PjrtRunner:
    """Builds the shard_map-jitted NEFF executor once; supports repeated
    dispatches with device-resident inputs for timing."""

    def __init__(self, nc):
        import jax
        import jax.numpy as jnp
        import concourse.mybir as mybir
        from concourse import bass2jax
        from jax.sharding import Mesh, NamedSharding, PartitionSpec
        from jax.experimental.shard_map import shard_map

        bass2jax.install_neuronx_cc_hook()
        self.jax = jax
        self.jnp = jnp
        in_names: list[str] = []
        out_names: list[str] = []
        out_avals = []
        out_shapes = []
        partition_name = (
            nc.partition_id_tensor.name if nc.partition_id_tensor else None
        )
        for alloc in nc.m.functions[0].allocations:
            if not isinstance(alloc, mybir.MemoryLocationSet):
                continue
            name = alloc.memorylocations[0].name
            if alloc.kind == "ExternalInput":
                if name != partition_name:
                    in_names.append(name)
            elif alloc.kind == "ExternalOutput":
                shape = tuple(alloc.tensor_shape)
                dtype = mybir.dt.np(alloc.dtype)
                out_names.append(name)
                out_avals.append(jax.core.ShapedArray(shape, dtype))
                out_shapes.append((shape, dtype))
        self.in_names = in_names
        self.out_names = out_names
        self.out_shapes = out_shapes
        n_params = len(in_names)
        n_outs = len(out_names)
        all_names = in_names + out_names
        if partition_name is not None:
            all_names = all_names + [partition_name]

        def _body(*args):
            operands = list(args)
            if partition_name is not None:
                operands.append(bass2jax.partition_id_tensor())
            outs = bass2jax._bass_exec_p.bind(
                *operands,
                out_avals=tuple(out_avals),
                in_names=tuple(all_names),
                out_names=tuple(out_names),
                lowering_input_output_aliases=(),
                sim_require_finite=True,
                sim_require_nnan=True,
                nc=nc,
            )
            return tuple(outs)

        devices = jax.devices()[:NCORES]
        self.mesh = Mesh(np.asarray(devices), ("core",))
        spec = PartitionSpec("core")
        self.sharding = NamedSharding(self.mesh, spec)
        self.sharded = jax.jit(
            shard_map(
                _body,
                mesh=self.mesh,
                in_specs=(spec,) * (n_params + n_outs),
                out_specs=(spec,) * n_outs,
                check_rep=False,
            ),
            donate_argnums=tuple(range(n_params, n_params + n_outs)),
            keep_unused=True,
        )

        def _mk_zeros():
            return tuple(
                jnp.zeros((NCORES * s[0], *s[1:]), d) for s, d in out_shapes
            )

        self.zeros_fn = jax.jit(
            _mk_zeros, out_shardings=(self.sharding,) * n_outs
        )

    def _stage_inputs(self, in_maps):
        concat = [
            np.concatenate(
                [np.asarray(in_maps[c][n]) for c in range(NCORES)], axis=0
            )
            for n in self.in_names
        ]
        return [self.jax.device_put(a, self.sharding) for a in concat]

    def _dispatch(self, staged):
        zeros = self.zeros_fn()
        outs = self.sharded(*staged, *zeros)
        self.jax.block_until_ready(outs)
        return outs

    def run(self, in_maps):
        staged = self._stage_inputs(in_maps)
        outs = self._dispatch(staged)
        res = []
        for c in range(NCORES):
            m = {}
            for i, n in enumerate(self.out_names):
                s, d = self.out_shapes[i]
                m[n] = np.asarray(outs[i]).reshape(NCORES, *s)[c]
            res.append(m)
        return res

    def timed(self, in_maps, iters=10):
        import time

        staged = self._stage_inputs(in_maps)
        self._dispatch(staged)  # warm
        walls = []
        for _ in range(iters):
            zeros = self.zeros_fn()
            self.jax.block_until_ready(zeros)
            t0 = time.perf_counter()
            outs = self.sharded(*staged, *zeros)
            self.jax.block_until_ready(outs)
            walls.append(time.perf_counter() - t0)
        tiny = self.jax.device_put(
            np.zeros((NCORES, 8), np.float32), self.sharding
        )
        base_fn = self.jax.jit(lambda a: a + 1.0)
        self.jax.block_until_ready(base_fn(tiny))
        bases = []
        for _ in range(iters):
            t0 = time.perf_counter()
            self.jax.block_until_ready(base_fn(tiny))
            bases.append(time.perf_counter() - t0)
        print(
            f"kernel walls min/med: {min(walls)*1e3:.2f}/"
            f"{np.median(walls)*1e3:.2f} ms; "
            f"baseline min/med: {min(bases)*1e3:.2f}/"
            f"{np.median(bases)*1e3:.2f} ms"
        )
        return max(float(np.median(walls) - np.median(bases)), 0.0) * 1e9


def _get_runner(nc):
    r = _RUNNER_CACHE.get(id(nc))
    if r is None:
        r = _PjrtRunner(nc)
        _RUNNER_CACHE[id(nc)] = r
    return r


def time_kernel(inputs=None, iters=8):
    assert _LAST is not None, "call kernel() first"
    nc, in_maps = _LAST
    return _get_runner(nc).timed(in_maps, iters=iters)

